# revision 4
# baseline (speedup 1.0000x reference)
"""Trainium2 Bass kernel for nn_Classifier_87256555586283 (KAN 2-layer MLP).

Math: kan_linear(x) = silu(x) @ base_w.T + einsum('nig,oig->no', B(x), spline_w*scaler)
with B(x) the 8 cubic B-spline bases on a uniform grid; mid nonlinearity is
exact (erf) gelu; layer 2 maps 768 -> 2.

Reformulation: with x' = min(x, 2.2), the 8 bases are an exact linear combo of
11 one-sided cubes relu(x' - t_s)^3, t_s = (s-5.5)/2.5 (2.5^3 and the 8->11
basis transform fold into the weights host-side).  Each layer is 12 elementwise
planes (silu + 11 cubes) feeding one K=12*768 contraction.

Precision: the cube basis is ill-conditioned (cube values up to 1331 cancel to
basis values <= 0.67), amplifying independent per-term rounding ~500x.  Planes
s=0..6 therefore use the fp16 hi/lo 3-matmul scheme (Wh@Ph + Wh@Pl + Wl@Ph);
planes s=7..10 (cube values <= 64) and the silu plane are single fp16 matmuls.
26 matmul units per 128-feature chunk vs 36 for all-hi/lo.  Host-simulated
end-to-end rel err ~7e-3 vs the 2e-2 gate.

Structure per core (data-parallel over tokens, 2048/core, no collectives):
single pass over K; hi+single weights SBUF-resident (108KB/partition); lo
weights streamed from DRAM per (tb, ic).  PSUM: 6 banks accumulate the 768-dim
L1 output per 512-token block; drain = ACT fp16 copy, gelu = erf + stt; L2
packs [128,2] matmuls 3-per-bank via tile_position col groups.  One ACT table
set (sigmoid_and_others: sigmoid, square, erf) - no set switches.  L2 blocks of
token-block N are emitted between L1 blocks of token-block N+1 so the PE and
the elementwise engines both stay busy (software pipelining).
"""

import math

import numpy as np

# problem constants (hardcoded per contract)
B, S, H, L = 32, 512, 768, 2
NTOK = B * S            # 16384
NCORES = 8
TPC = NTOK // NCORES    # 2048 tokens per core
NTB = 512               # token block (PSUM bank = 512 fp32)
NNT = TPC // NTB        # 4
NIC = H // 128          # 6
NOC = H // 128          # 6
NS = 11                 # cube planes
S3 = (0, 1, 2, 3, 4, 5, 6)   # hi/lo (3-matmul) planes
S1 = (7, 8, 9, 10)           # single-fp16 planes
TVALS = tuple((s - 5.5) / 2.5 for s in range(NS))
CUBE_SCALE = 15.625     # 2.5^3 folded into spline weights

# resident stationary chunk order per ic: b, s7..10 (single), hi(s0..6)
RES_UNITS = ("b", 7, 8, 9, 10, 0, 1, 2, 3, 4, 5, 6)
NRES = len(RES_UNITS)               # 12
NLO = len(S3)                       # 7
WHI_COLS = NIC * NRES * NOC * 128   # 55296
WLO_COLS = NIC * NLO * NOC * 128    # 32256
LO_SZ = NLO * NOC * 128             # 5376 cols per ic
NU2 = 1 + len(S1) + 3 * len(S3)     # 26 matmul units per ic
NK2 = NIC * NU2                     # 156 accumulating matmuls per (tb, bank)

# engine routing: 'v' = DVE stt path, 'g' = GPSIMD tensor_tensor path (via a
# relu'd d tile); ph cast and pl subtract engines likewise.
ROUTE = {0: "v", 1: "g", 2: "v", 3: "g", 4: "v", 5: "g", 6: "v",
         7: "v", 8: "g", 9: "v", 10: "g"}
PH_ENG = {0: "a", 1: "v", 2: "a", 3: "v", 4: "a", 5: "v", 6: "a"}
PL_ENG = {0: "g", 1: "v", 2: "g", 3: "v", 4: "g", 5: "v", 6: "g"}

_PROGRAM = None


def _basis_transform():
    """(8, 11) matrix C with bases[g] = sum_s C[g, s] * relu(u - s)^3 on [0, 11]."""
    C = np.zeros((8, 12), np.float64)
    for g in range(8):
        for r in range(5):
            C[g, g + r] = ((-1) ** r) * math.comb(4, r) / 6.0
    return C[:, :11]


def _fold_spline(spline_w, scaler):
    C = _basis_transform()
    return np.einsum(
        "oig,gs->ois",
        spline_w.astype(np.float64) * scaler[..., None].astype(np.float64),
        C,
    ) * CUBE_SCALE  # float64 (out, in, 11)


def _pack_weights(base_w1, spline_w1, scaler1, base_w2, spline_w2, scaler2):
    W1p = _fold_spline(spline_w1, scaler1)      # (768, 768, 11)
    hi1 = W1p.astype(np.float16)
    lo1 = (W1p - hi1.astype(np.float64)).astype(np.float16)
    b1h = base_w1.astype(np.float16)

    whi = np.empty((128, WHI_COLS), np.float16)
    wlo = np.empty((128, WLO_COLS), np.float16)
    for ic in range(NIC):
        isl = slice(ic * 128, (ic + 1) * 128)
        for j, t in enumerate(RES_UNITS):
            for oc in range(NOC):
                osl = slice(oc * 128, (oc + 1) * 128)
                blk = b1h[osl, isl].T if t == "b" else hi1[osl, isl, t].T
                col = ((ic * NRES + j) * NOC + oc) * 128
                whi[:, col : col + 128] = blk
        for si, s in enumerate(S3):
            for oc in range(NOC):
                osl = slice(oc * 128, (oc + 1) * 128)
                col = ((ic * NLO + si) * NOC + oc) * 128
                wlo[:, col : col + 128] = lo1[osl, isl, s].T

    W2p = _fold_spline(spline_w2, scaler2)      # (2, 768, 11)
    hi2 = W2p.astype(np.float16)
    lo2 = (W2p - hi2.astype(np.float64)).astype(np.float16)
    b2h = base_w2.astype(np.float16)

    w2a = np.zeros((128, NK2 * 2), np.float16)
    for ic in range(NIC):
        isl = slice(ic * 128, (ic + 1) * 128)
        chunks = [b2h[:, isl].T]
        for s in S1:
            chunks.append(hi2[:, isl, s].T)
        for s in S3:
            chunks.append(hi2[:, isl, s].T)   # hh
            chunks.append(hi2[:, isl, s].T)   # hl
            chunks.append(lo2[:, isl, s].T)   # lh
        for u, blk in enumerate(chunks):
            k2 = ic * NU2 + u
            w2a[:, k2 * 2 : k2 * 2 + 2] = blk
    w2o = np.zeros((128, 2), np.float16)
    for cg in range(3):
        for l in range(L):
            w2o[32 * cg + l, l] = 1.0
    return (
        np.ascontiguousarray(whi),
        np.ascontiguousarray(wlo),
        np.ascontiguousarray(w2a),
        np.ascontiguousarray(w2o),
    )


def _build_program():
    import concourse.bass as bass  # noqa: F401
    import concourse.tile as tile
    from concourse import bacc, mybir

    f32 = mybir.dt.float32
    f16 = mybir.dt.float16
    A = mybir.ActivationFunctionType
    OP = mybir.AluOpType

    nc = bacc.Bacc(None, target_bir_lowering=False, debug=False)
    # ACT Square biases (-t_s) need registered const APs ([128,1] SBUF)
    for s in range(NS):
        val = float(-TVALS[s])
        if (f32, val) not in nc.const_aps.aps:
            t = nc.alloc_sbuf_tensor(f"constb-{s}", [128, 1], f32)
            nc.gpsimd.memset(t.ap(), val)
            nc.const_aps.aps[(f32, val)] = t.ap()
    nc.all_engine_barrier()

    xT_d = nc.dram_tensor("xT16", [H, TPC], f16, kind="ExternalInput")
    whi_d = nc.dram_tensor("w1hi", [128, WHI_COLS], f16, kind="ExternalInput")
    wlo_d = nc.dram_tensor("w1lo", [128, WLO_COLS], f16, kind="ExternalInput")
    w2a_d = nc.dram_tensor("w2a", [128, NK2 * 2], f16, kind="ExternalInput")
    w2o_d = nc.dram_tensor("w2o", [128, 2], f16, kind="ExternalInput")
    out_d = nc.dram_tensor("outT", [L, TPC], f32, kind="ExternalOutput")

    with tile.TileContext(nc) as tc:
        from contextlib import ExitStack

        with ExitStack() as ctx:
            wpool = ctx.enter_context(tc.tile_pool(name="wpool", bufs=1))
            whi = wpool.tile([128, WHI_COLS], f16, name="whi_sb")
            nchunk = 12
            csz = WHI_COLS // nchunk
            for k in range(nchunk):
                nc.sync.dma_start(
                    whi[:, k * csz : (k + 1) * csz],
                    whi_d[:, k * csz : (k + 1) * csz],
                )
            w2a = wpool.tile([128, NK2 * 2], f16, name="w2a_sb")
            w2o = wpool.tile([128, 2], f16, name="w2o_sb")
            nc.sync.dma_start(w2a[:], w2a_d[:])
            nc.sync.dma_start(w2o[:], w2o_d[:])

            psum = ctx.enter_context(tc.tile_pool(name="psum", bufs=1, space="PSUM"))
            l1ps = [
                psum.tile([128, NTB], f32, name=f"l1ps{oc}", tag=f"l1ps{oc}")
                for oc in range(NOC)
            ]
            l2ps = psum.tile([128, NTB], f32, name="l2ps", tag="l2ps")
            fps = psum.tile([128, NTB], f32, name="fps", tag="fps")
            nc.vector.memset(l2ps[:], 0.0)

            sm = ctx.enter_context(tc.tile_pool(name="sm", bufs=1))
            lopool = ctx.enter_context(tc.tile_pool(name="lopool", bufs=1))
            eng = {"v": nc.vector, "g": nc.gpsimd}

            lo_tiles = []

            def fetch_lo(ic):
                t = lopool.tile([128, LO_SZ], f16, name="wlo_sb", tag="wlo", bufs=2)
                nc.sync.dma_start(t[:], wlo_d[:, ic * LO_SZ : (ic + 1) * LO_SZ])
                lo_tiles.append(t)

            fetch_lo(0)

            def whi_ap(ic, unit, oc):
                col = ((ic * NRES + RES_UNITS.index(unit)) * NOC + oc) * 128
                return whi[:, col : col + 128]

            def build_planes(src16, mm_unit):
                """11 cube planes of a clipped fp16 input tile, in MM-unit
                order; hi/lo split for S3.  mm_unit(kind, s, moving_tile)."""
                for s in S1:
                    e = sm.tile([128, NTB], f32, name=f"e{s}", tag="e", bufs=3)
                    nc.scalar.activation(e[:], src16[:], A.Square, bias=-TVALS[s])
                    p = sm.tile([128, NTB], f16, name=f"p{s}", tag="p16", bufs=5)
                    if ROUTE[s] == "v":
                        c = sm.tile([128, NTB], f16, name="c", tag="c", bufs=2)
                        nc.vector.scalar_tensor_tensor(
                            c[:], src16[:], -TVALS[s], e[:], OP.add, OP.mult
                        )
                        nc.vector.tensor_scalar(p[:], c[:], 0.0, None, OP.max)
                    else:
                        d = sm.tile([128, NTB], f32, name="d", tag="d", bufs=2)
                        nc.vector.tensor_scalar(
                            d[:], src16[:], -TVALS[s], 0.0, OP.add, OP.max
                        )
                        nc.gpsimd.tensor_tensor(p[:], d[:], e[:], OP.mult)
                    mm_unit("s", s, p)
                for s in S3:
                    e = sm.tile([128, NTB], f32, name=f"e{s}", tag="e", bufs=3)
                    nc.scalar.activation(e[:], src16[:], A.Square, bias=-TVALS[s])
                    pg = sm.tile([128, NTB], f32, name="pg", tag="pg", bufs=2)
                    if ROUTE[s] == "v":
                        pu = sm.tile([128, NTB], f32, name="pu", tag="pu", bufs=2)
                        nc.vector.scalar_tensor_tensor(
                            pu[:], src16[:], -TVALS[s], e[:], OP.add, OP.mult
                        )
                        nc.vector.tensor_scalar(pg[:], pu[:], 0.0, None, OP.max)
                    else:
                        d = sm.tile([128, NTB], f32, name="d", tag="d", bufs=2)
                        nc.vector.tensor_scalar(
                            d[:], src16[:], -TVALS[s], 0.0, OP.add, OP.max
                        )
                        nc.gpsimd.tensor_tensor(pg[:], d[:], e[:], OP.mult)
                    ph = sm.tile([128, NTB], f16, name="ph", tag="ph", bufs=4)
                    if PH_ENG[s] == "a":
                        nc.scalar.copy(ph[:], pg[:])
                    else:
                        nc.vector.tensor_copy(ph[:], pg[:])
                    pl = sm.tile([128, NTB], f16, name="pl", tag="pl", bufs=4)
                    eng[PL_ENG[s]].tensor_tensor(pl[:], pg[:], ph[:], OP.subtract)
                    mm_unit("hh", s, ph)
                    mm_unit("hl", s, pl)
                    mm_unit("lh", s, ph)

            h16s = {}       # tb -> list of 6 fp16 L1-output tiles
            l1_idx = {}     # tb -> [unit counter]
            l2_idx = {}

            def emit_l1_block(tb, ic):
                tsl = slice(tb * NTB, (tb + 1) * NTB)
                lin = tb * NIC + ic
                if lin + 1 < NNT * NIC:
                    fetch_lo((ic + 1) % NIC)
                lo = lo_tiles[lin]
                idx = l1_idx.setdefault(tb, [0])

                def mm_unit(kind, s, mv):
                    u = idx[0]
                    idx[0] += 1
                    for oc in range(NOC):
                        if kind == "b":
                            w = whi_ap(ic, "b", oc)
                        elif kind in ("s", "hh", "hl"):
                            w = whi_ap(ic, s, oc)
                        else:  # lh
                            col = (S3.index(s) * NOC + oc) * 128
                            w = lo[:, col : col + 128]
                        nc.tensor.matmul(
                            l1ps[oc][:], w, mv[:],
                            start=(u == 0), stop=(u == NK2 - 1),
                        )

                xs = sm.tile([128, NTB], f16, name="xs", tag="xs", bufs=3)
                nc.sync.dma_start(xs[:], xT_d[ic * 128 : (ic + 1) * 128, tsl])
                xp = sm.tile([128, NTB], f16, name="xp", tag="xp", bufs=3)
                nc.vector.tensor_scalar(xp[:], xs[:], 2.2, None, OP.min)
                # silu base plane: x * sigmoid(x)
                sg = sm.tile([128, NTB], f16, name="sg", tag="sg", bufs=2)
                nc.scalar.activation(sg[:], xs[:], A.Sigmoid)
                sp = sm.tile([128, NTB], f16, name="sp", tag="p16", bufs=5)
                nc.vector.tensor_tensor(sp[:], xs[:], sg[:], OP.mult)
                mm_unit("b", "b", sp)
                build_planes(xp, mm_unit)

            def emit_l1_drain(tb):
                # drain on ACT: frees the banks fast, h16 feeds gelu/L2
                hs = []
                for oc in range(NOC):
                    h = sm.tile([128, NTB], f16, name=f"h{oc}", tag=f"h{oc}",
                                bufs=2)
                    nc.scalar.copy(h[:], l1ps[oc][:])
                    hs.append(h)
                h16s[tb] = hs

            def emit_l2_block(tb, j):
                idx = l2_idx.setdefault(tb, [0])

                def mm_unit(kind, s, mv):
                    k2 = idx[0]
                    idx[0] += 1
                    cg = k2 % 3
                    nc.tensor.matmul(
                        l2ps[32 * cg : 32 * cg + 2, :],
                        w2a[:, k2 * 2 : k2 * 2 + 2],
                        mv[:],
                        start=(k2 < 3), stop=(k2 >= NK2 - 3),
                        tile_position=(0, 32 * cg),
                    )

                h = h16s[tb][j]
                # exact gelu: z = h*(1+erf(h/sqrt(2))) = 2*gelu(h)
                er = sm.tile([128, NTB], f16, name="er", tag="er", bufs=2)
                nc.scalar.activation(
                    er[:], h[:], A.Erf, bias=0.0, scale=0.7071067811865476
                )
                z = sm.tile([128, NTB], f16, name="z", tag="z", bufs=2)
                nc.vector.scalar_tensor_tensor(
                    z[:], er[:], 1.0, h[:], OP.add, OP.mult
                )
                # y' = min(0.5 z, 2.2) feeds the cube planes
                yp = sm.tile([128, NTB], f16, name="yp", tag="yp", bufs=2)
                nc.vector.tensor_scalar(yp[:], z[:], 0.5, 2.2, OP.mult, OP.min)
                # silu(y) = 0.5*sigmoid(0.5 z)*z
                sg2 = sm.tile([128, NTB], f16, name="sg2", tag="sg", bufs=2)
                nc.scalar.activation(sg2[:], z[:], A.Sigmoid, scale=0.5)
                sp2 = sm.tile([128, NTB], f16, name="sp2", tag="p16", bufs=5)
                nc.vector.scalar_tensor_tensor(
                    sp2[:], sg2[:], 0.5, z[:], OP.mult, OP.mult
                )
                mm_unit("b", "b", sp2)
                build_planes(yp, mm_unit)

            def emit_l2_finish(tb):
                tsl = slice(tb * NTB, (tb + 1) * NTB)
                cp = sm.tile([128, NTB], f16, name="cp", tag="cp", bufs=2)
                nc.vector.tensor_copy(cp[:], l2ps[:])
                nc.tensor.matmul(fps[0:L, :], w2o[:], cp[:], start=True, stop=True)
                ob = sm.tile([L, NTB], f32, name="ob", tag="ob", bufs=2)
                nc.vector.tensor_copy(ob[:], fps[0:L, :])
                nc.sync.dma_start(out_d[:, tsl], ob[:])
                h16s.pop(tb)

            # software pipeline: L2 blocks of tb-1 interleave between L1
            # blocks of tb, so PE order is L1(tb,ic), L2(tb-1,ic), ...
            pending = None
            for tb in range(NNT):
                for ic in range(NIC):
                    emit_l1_block(tb, ic)
                    if pending is not None:
                        emit_l2_block(pending, ic)
                        if ic == NIC - 1:
                            emit_l2_finish(pending)
                emit_l1_drain(tb)
                pending = tb
            for j in range(NIC):
                emit_l2_block(pending, j)
            emit_l2_finish(pending)

    nc.compile()
    return nc


def _get_program():
    global _PROGRAM
    if _PROGRAM is None:
        _PROGRAM = _build_program()
    return _PROGRAM


def run(hidden, base_w1, spline_w1, scaler1, base_w2, spline_w2, scaler2, **kw):
    """Builds inputs, runs the SPMD kernel on 8 cores. Returns (output, results)."""
    from concourse.bass_utils import run_bass_kernel_spmd

    nc = _get_program()
    x = np.asarray(hidden, dtype=np.float32).reshape(NTOK, H)
    whi, wlo, w2a, w2o = _pack_weights(
        np.asarray(base_w1), np.asarray(spline_w1), np.asarray(scaler1),
        np.asarray(base_w2), np.asarray(spline_w2), np.asarray(scaler2),
    )
    in_maps = []
    for c in range(NCORES):
        xT16 = np.ascontiguousarray(x[c * TPC : (c + 1) * TPC].T.astype(np.float16))
        in_maps.append(
            {"xT16": xT16, "w1hi": whi, "w1lo": wlo, "w2a": w2a, "w2o": w2o}
        )
    res = run_bass_kernel_spmd(nc, in_maps, list(range(NCORES)), **kw)
    outs = [r["outT"].T for r in res.results]  # each (2048, 2)
    out = np.concatenate(outs, axis=0).reshape(B, S, L).astype(np.float32)
    return out, res


def kernel(**inputs):
    out, _ = run(**inputs)
    return out
